# revision 27
# baseline (speedup 1.0000x reference)
"""Distributed Trainium2 Bass kernel for nn_Attention (GQA attention + LoRA + RoPE).

Sharding: tensor-parallel over heads across 8 NeuronCores.
  - core c owns Q heads 4c..4c+3 and KV head c (GQA group).
  - wq/wk/wv column-sharded; wo COLUMN-sharded (each core computes a
    512-column slice of the output over the full 4096 contraction, fed by an
    AllGather of all cores' per-head attention outputs).
  - LoRA is folded into wq/wv on the host (x@wq + (x@A)@B == x@(wq + A@B)).
  - 1/sqrt(HD) folded into wq.
  - RoPE pair permutation folded into wq/wk column order: within each head the
    even dims come first, odd dims second, so on-device RoPE is plain
    elementwise math on partition halves.

Schedule: software-pipelined emission.  The tensor queue is in-order, so
attention matmuls (which wait on scalar-engine exp) are interleaved at
emission time with the next projection quarter / wo matmuls; exp latency
hides under independent matmuls instead of head-of-line blocking.
DMAs are batched (multi-k-tile, contiguous per partition) so the issue
queues don't bottleneck.  Causal masking of diagonal tiles is a vector
multiply with precomputed 0/1 tiles (gpsimd must stay free: it hosts the
blocking AllGather waits).  Output is written bf16 and widened on host.
"""

import sys
import types

import numpy as np
import ml_dtypes

import concourse.bass as bass
from concourse import bacc
import concourse.mybir as mybir
import concourse.tile as tile
from concourse.bass_utils import run_bass_kernel_spmd
from concourse.masks import make_identity


def _ensure_axon_hooks():
    """run_bass_kernel_spmd(trace=True) imports antenv.axon_hooks, which some
    images lack; install a no-op shim so a BASS_TRACE env var can't crash us."""
    try:
        import antenv
    except ImportError:
        return
    if "antenv.axon_hooks" in sys.modules:
        return
    try:
        from antenv import axon_hooks  # noqa: F401
        return
    except ImportError:
        pass
    mod = types.ModuleType("antenv.axon_hooks")
    mod._hook = None
    mod.set_axon_ntff_profile_hook = lambda h: setattr(mod, "_hook", h)
    mod.get_axon_ntff_profile_hook = lambda: mod._hook
    sys.modules["antenv.axon_hooks"] = mod
    antenv.axon_hooks = mod


_ensure_axon_hooks()

B, S, D = 2, 1024, 4096
H, KVH, HD = 32, 8, 128
NCORES = 8
HPC = H // NCORES            # 4 q heads per core
QCOLS = HPC * HD             # 512
T = B * S                    # 2048
P = 128
KT = D // P                  # 32 k tiles
KB = 4                       # k tiles per DMA block
NKB = KT // KB               # 8 blocks
NQ = 4                       # token quarters (512 tokens each)
QW = T // NQ                 # 512
SQC = 2                      # sq chunks per batch
STB = S // P                 # 8 st blocks per batch

FP32 = mybir.dt.float32
BF16 = mybir.dt.bfloat16
EXP = mybir.ActivationFunctionType.Exp

_COMPILED = {}
LAST_RESULTS = None


def _st_list(variant, sqc):
    """st blocks contributing to sq chunk sqc.
    Returns (st, flag): causal -> flag means diagonal-crossing (needs
    triangular zeroing of probs); general -> flag means mask preload."""
    out = []
    for st in range(STB):
        if variant == "causal":
            if st >= 4 * sqc + 4:
                continue  # fully masked
            flag = st >= 4 * sqc
        elif variant == "nomask":
            flag = False
        else:
            flag = True
        out.append((st, flag))
    return out


_DONE = object()


def _merge(gen_a, gen_b, offset=3, ratio_b=1.0):
    """Round-robin merge of two emission generators.  gen_a (dense matmul
    work) leads by `offset` units so gen_b's cross-engine deps are ready;
    ratio_b units of gen_b (cumulative, may be fractional) are pulled per
    gen_a unit."""
    for _ in range(offset):
        next(gen_a, _DONE)
    a_done = b_done = False
    owed = 0.0
    while not (a_done and b_done):
        if not a_done:
            a_done = next(gen_a, _DONE) is _DONE
        owed += ratio_b
        while owed >= 1.0:
            owed -= 1.0
            if not b_done:
                b_done = next(gen_b, _DONE) is _DONE


def _drain(gen):
    for _ in gen:
        pass


def _build(variant, debug=False):
    nc = bacc.Bacc(None)

    xt_e = nc.declare_dram_parameter("xt", [P, NQ, KT, QW], BF16, isOutput=False)
    wq_e = nc.declare_dram_parameter("wq", [P, KT, QCOLS], BF16, isOutput=False)
    wk_e = nc.declare_dram_parameter("wk", [P, KT, HD], BF16, isOutput=False)
    wv_e = nc.declare_dram_parameter("wv", [P, KT, HD], BF16, isOutput=False)
    wo_e = nc.declare_dram_parameter("wo", [P, KT, QCOLS], BF16, isOutput=False)
    # cos: [c; c] duplicated halves.  sin: [s; -s] (negated bottom half).
    cos_e = nc.declare_dram_parameter("cos", [P, T], BF16, isOutput=False)
    sin_e = nc.declare_dram_parameter("sin", [P, T], BF16, isOutput=False)
    if variant == "general":
        mk_e = nc.declare_dram_parameter("mk", [P, STB, S], BF16, isOutput=False)
    out_e = nc.declare_dram_parameter("out", [QCOLS, T], BF16, isOutput=True)

    general = variant == "general"
    xt_bufs = 4 if general else 6
    pr_bufs = 8 if general else 10

    with tile.TileContext(nc) as tc:
        with (
            tc.tile_pool(name="wpool", bufs=1) as wpool,
            tc.tile_pool(name="cst", bufs=1) as cst,
            tc.tile_pool(name="persist", bufs=1) as persist,
            tc.tile_pool(name="xt", bufs=xt_bufs) as xtp,
            tc.tile_pool(name="ev", bufs=4) as evp,
            tc.tile_pool(name="rt", bufs=2) as rtp,
            tc.tile_pool(name="probs", bufs=pr_bufs) as prp,
            tc.tile_pool(name="dsum", bufs=5) as dsp,
            tc.tile_pool(name="misc", bufs=2) as mip,
            tc.tile_pool(name="ag", bufs=3) as agp,
            tc.tile_pool(name="ow", bufs=4) as owp,
            tc.tile_pool(name="ps", bufs=1, space="PSUM") as psp,
            tc.tile_pool(name="dram", bufs=1, space="DRAM") as dram,
        ):
            # ---- resident weights / constants ----
            wq_sb = wpool.tile([P, KT, QCOLS], BF16, name="wq_sb")
            wk_sb = wpool.tile([P, KT, HD], BF16, name="wk_sb")
            wv_sb = wpool.tile([P, KT, HD], BF16, name="wv_sb")
            wo_sb = wpool.tile([P, KT, QCOLS], BF16, name="wo_sb")
            cos_sb = wpool.tile([P, T], BF16, name="cos_sb")
            sin_sb = wpool.tile([P, T], BF16, name="sin_sb")
            if general:
                mk_sb = wpool.tile([P, STB, S], BF16, name="mk_sb")

            ident = cst.tile([P, P], BF16, name="ident")
            make_identity(nc, ident)
            ones_sq = cst.tile([P, P], BF16, name="ones_sq")
            nc.vector.memset(ones_sq[:], 1.0)
            # Causal diagonal mask for a PAIRED probs tile [st2c | st2c+1] over
            # one 256-query chunk: left half keeps q >= p, right keeps
            # q >= p+128.  The relative structure is identical for every
            # chunk, so one tile serves all diagonals.
            CW = QW // 2
            dmask2 = cst.tile([P, QW], BF16, name="dmask2")
            if variant == "causal":
                nc.gpsimd.memset(dmask2[:], 1.0)
                for j in range(2):
                    nc.gpsimd.affine_select(
                        out=dmask2[:, j * CW:(j + 1) * CW],
                        in_=dmask2[:, j * CW:(j + 1) * CW],
                        compare_op=mybir.AluOpType.is_ge,
                        fill=0.0, base=-j * P, channel_multiplier=-1,
                        pattern=[[1, CW]])

            # ---- persistent activations ----
            q_rot = [[persist.tile([P, S], BF16, name=f"q{h}_{b}")
                      for b in range(B)] for h in range(HPC)]
            k_rot = [persist.tile([P, S], BF16, name=f"k{b}") for b in range(B)]
            v_sb = [persist.tile([P, STB, P], BF16, name=f"v{b}") for b in range(B)]
            attn = [[persist.tile([P, S], BF16, name=f"attn{h}_{b}")
                     for b in range(B)] for h in range(HPC)]

            # one AllGather per (batch, token-half): each triggers as soon as
            # its attention chunk is done, so no gather is ever on the
            # critical path of the wo matmuls that consume it.
            ag_in = [[dram.tile([HPC * P, QW], BF16, name=f"agin{b}_{hf}")
                      for hf in range(SQC)] for b in range(B)]
            ag_out = [[dram.tile([H * P, QW], BF16, addr_space="Shared",
                                 name=f"agout{b}_{hf}")
                       for hf in range(SQC)] for b in range(B)]
            ag_r = [[ag_out[b][hf].rearrange("(k p) t -> p k t", p=P)
                     for hf in range(SQC)] for b in range(B)]

            def pp(n=1):
                return [psp.tile([P, QW], FP32, name="pp", tag="pp", bufs=6)
                        for _ in range(n)]

            def ap(shape=(P, QW), dtype=FP32):
                return psp.tile(list(shape), dtype, name="ap", tag="ap", bufs=2)

            xt_blocks = {}

            def issue_xt(qx, kb):
                t = xtp.tile([P, KB, QW], BF16, name="xt")
                nc.sync.dma_start(t[:], xt_e[:, qx, kb * KB:(kb + 1) * KB, :])
                xt_blocks[(qx, kb)] = t

            def rope(dst, dst_off, src_bf, qoff):
                """RoPE on split layout (a=0:64, b=64:128).
                p1 = [a*c; b*c];  p2 = [a*s; -b*s];  swap halves of p2;
                dst = p1 + p2sw = [a*c - b*s; a*s + b*c]."""
                c = cos_sb[:, qoff:qoff + QW]
                s = sin_sb[:, qoff:qoff + QW]
                p1 = rtp.tile([P, QW], BF16, name="p1")
                p2 = rtp.tile([P, QW], BF16, name="p2")
                p2sw = rtp.tile([P, QW], BF16, name="p2sw")
                nc.vector.tensor_mul(p1[:], src_bf[:], c)
                nc.vector.tensor_mul(p2[:], src_bf[:], s)
                nc.vector.tensor_copy(p2sw[0:64, :], p2[64:128, :])
                nc.vector.tensor_copy(p2sw[64:128, :], p2[0:64, :])
                nc.vector.tensor_add(dst[:, dst_off:dst_off + QW], p1[:], p2sw[:])

            def preload():
                """Startup DMA order: weights arrive in k order just ahead of
                the k-loop's consumption, so no single large transfer delays
                an early k-tile."""
                def wblk(w_sb, w_e, k0, k1):
                    nc.sync.dma_start(w_sb[:, k0:k1, :], w_e[:, k0:k1, :])

                # wk/wv whole (8 KB contiguous per partition — best DMA line
                # size); wq/xt per 4-k block so arrival tracks consumption.
                nc.sync.dma_start(wk_sb[:], wk_e[:])
                issue_xt(0, 0)
                nc.sync.dma_start(wv_sb[:], wv_e[:])
                wblk(wq_sb, wq_e, 0, KB)
                nc.gpsimd.dma_start(cos_sb[:], cos_e[:])
                nc.gpsimd.dma_start(sin_sb[:], sin_e[:])
                for kb in range(1, NKB):
                    wblk(wq_sb, wq_e, kb * KB, (kb + 1) * KB)
                    issue_xt(0, kb)
                if general:
                    nc.gpsimd.dma_start(mk_sb[:], mk_e[:])

            proj_ps = {}

            def proj_units(qx):
                """Yield after each k-tile's 6 matmuls.  Prefetches the next
                quarter's xt blocks as this quarter's are consumed."""
                toff = qx * QW
                psums = pp(6)
                proj_ps[qx] = psums
                for kb in range(NKB):
                    xt = xt_blocks.pop((qx, kb))
                    for j in range(KB):
                        k = kb * KB + j
                        rhs = xt[:, j, :]
                        for mb in range(6):
                            if mb == 0:
                                w = wk_sb[:, k, :]
                            elif mb == 1:
                                w = wv_sb[:, k, :]
                            else:
                                w = wq_sb[:, k, (mb - 2) * P:(mb - 1) * P]
                            nc.tensor.matmul(psums[mb][:], w, rhs,
                                             start=(k == 0), stop=(k == KT - 1))
                        yield
                    if qx + 1 < NQ:
                        issue_xt(qx + 1, kb)

            def finish(qx):
                """Evict + rope this quarter's projections.  Order: k first
                (scores need it), then q heads, then v (PV needs it last)."""
                b, boff = qx // 2, (qx % 2) * QW
                toff = qx * QW
                pk, pvv, *pq = proj_ps.pop(qx)
                ke = evp.tile([P, QW], BF16, name="ke", tag="qe")
                nc.scalar.copy(ke[:], pk[:])
                rope(k_rot[b], boff, ke, toff)
                for h in range(HPC):
                    qe = evp.tile([P, QW], BF16, name="qe", tag="qe")
                    nc.scalar.copy(qe[:], pq[h][:])
                    rope(q_rot[h][b], boff, qe, toff)
                ve = evp.tile([P, QW], BF16, name="ve", tag="qe")
                nc.scalar.copy(ve[:], pvv[:])
                for i in range(QW // P):
                    st = (qx % 2) * 4 + i
                    tp = ap((P, P), BF16)
                    nc.tensor.transpose(tp[:], ve[:, i * P:(i + 1) * P], ident[:])
                    nc.scalar.copy(v_sb[b][:, st, :], tp[:])

            def attn_units(b, sqc):
                """Yield at points where the tensor queue may have to wait on
                the scalar engine (exp) — merge partner fills the gaps.
                The softmax denominator is computed by tree-summing the probs
                tiles on the vector engine (keys are partition-aligned across
                st blocks), then ONE ones-matmul — instead of one matmul per
                tile."""
                sq0 = sqc * QW
                stl = _st_list(variant, sqc)
                n = len(stl)
                for h in range(HPC):
                    prtiles = []
                    sums = []

                    def vadd(x, y):
                        t = dsp.tile([P, QW], BF16, name="ds")
                        nc.vector.tensor_add(t[:], x[:], y[:])
                        return t

                    for i0 in range(0, n, 2):
                        for st, flag in stl[i0:i0 + 2]:
                            pss = ap()
                            preload_mk = general and flag
                            if preload_mk:
                                nc.tensor.matmul(pss[:], ident[:],
                                                 mk_sb[:, st, sq0:sq0 + QW],
                                                 start=True, stop=False)
                            nc.tensor.matmul(
                                pss[:], k_rot[b][:, st * P:(st + 1) * P],
                                q_rot[h][b][:, sq0:sq0 + QW],
                                start=(not preload_mk), stop=True)
                            pr = prp.tile([P, QW], BF16, name="pr", tag="pr")
                            nc.scalar.activation(pr[:], pss[:], EXP)
                            if flag and variant == "causal":
                                j = st - 4 * sqc
                                nc.vector.tensor_mul(pr[:], pr[:],
                                                     dmask[:, j, :])
                            prtiles.append(pr)
                        sums.append(vadd(prtiles[-2], prtiles[-1]))
                        yield
                    pso = ap()
                    psdb = ap()
                    for i0 in range(0, n, 2):
                        for i in range(i0, min(i0 + 2, n)):
                            st = stl[i][0]
                            nc.tensor.matmul(pso[:], v_sb[b][:, st, :],
                                             prtiles[i][:],
                                             start=(i == 0), stop=(i == n - 1))
                        if len(sums) > 1:
                            sums = sums[2:] + [vadd(sums[0], sums[1])]
                        yield
                    nc.tensor.matmul(psdb[:], ones_sq[:], sums[0][:],
                                     start=True, stop=True)
                    rb = mip.tile([P, QW], FP32, name="rb")
                    nc.vector.reciprocal_approx_fast(rb[:], psdb[:])
                    nc.vector.tensor_mul(attn[h][b][:, sq0:sq0 + QW],
                                         pso[:], rb[:])
                    nc.sync.dma_start(ag_in[b][sqc][h * P:(h + 1) * P, :],
                                      attn[h][b][:, sq0:sq0 + QW])
                    yield

            def attn_units_causal(b, half):
                """Block-exact causal attention at 256-query granularity.
                Two 256-wide score tiles for adjacent key blocks share one
                PSUM bank side-by-side, so exp still runs on full [128,512]
                tiles and the diagonal needs a single mask multiply.
                half 0 covers chunks 0-1 (keys st 0..3, from the even
                quarter); half 1 covers chunks 2-3 (st 0..7)."""
                for h in range(HPC):
                    for c in (2 * half, 2 * half + 1):
                        q0c = c * CW
                        qs = q_rot[h][b][:, q0c:q0c + CW]
                        npair = c + 1
                        prtiles = []
                        sums = []

                        def vadd(x, y):
                            t = dsp.tile([P, QW], BF16, name="ds")
                            nc.vector.tensor_add(t[:], x[:], y[:])
                            return t

                        for pi in range(npair):
                            pss = ap()
                            for sub in range(2):
                                st = 2 * pi + sub
                                nc.tensor.matmul(
                                    pss[:, sub * CW:(sub + 1) * CW],
                                    k_rot[b][:, st * P:(st + 1) * P], qs,
                                    start=True, stop=True)
                            pr = prp.tile([P, QW], BF16, name="pr", tag="pr")
                            nc.scalar.activation(pr[:], pss[:], EXP)
                            if pi == npair - 1:
                                nc.vector.tensor_mul(pr[:], pr[:], dmask2[:])
                            prtiles.append(pr)
                            yield
                        pso = ap((P, CW))
                        psdb = ap((P, CW))
                        for pi in range(npair):
                            for sub in range(2):
                                st = 2 * pi + sub
                                nc.tensor.matmul(
                                    pso[:], v_sb[b][:, st, :],
                                    prtiles[pi][:, sub * CW:(sub + 1) * CW],
                                    start=(pi == 0 and sub == 0),
                                    stop=(pi == npair - 1 and sub == 1))
                            if pi % 2 == 1:
                                sums.append(vadd(prtiles[pi - 1], prtiles[pi]))
                            yield
                        if npair % 2 == 1:
                            sums.append(prtiles[-1])
                        while len(sums) > 1:
                            sums = sums[2:] + [vadd(sums[0], sums[1])]
                        sh = dsp.tile([P, CW], BF16, name="dsh")
                        nc.vector.tensor_add(sh[:], sums[0][:, 0:CW],
                                             sums[0][:, CW:QW])
                        nc.tensor.matmul(psdb[:], ones_sq[:], sh[:],
                                         start=True, stop=True)
                        rb = mip.tile([P, CW], FP32, name="rbc", tag="rbc")
                        nc.vector.reciprocal_approx_fast(rb[:], psdb[:])
                        nc.vector.tensor_mul(attn[h][b][:, q0c:q0c + CW],
                                             pso[:], rb[:])
                        yield
                    nc.sync.dma_start(
                        ag_in[b][half][h * P:(h + 1) * P, :],
                        attn[h][b][:, half * QW:(half + 1) * QW])
                    yield

            def attn_gen(b, half):
                if variant == "causal":
                    return attn_units_causal(b, half)
                return attn_units(b, half)

            def gather(b, hf):
                nc.gpsimd.collective_compute(
                    "AllGather", mybir.AluOpType.bypass,
                    ins=[ag_in[b][hf][:].opt()],
                    outs=[ag_out[b][hf][:].opt()],
                    replica_groups=[list(range(NCORES))],
                )

            def wo_load():
                # gpsimd queue is idle here (and only hosts the collectives
                # later), so weight streaming doesn't contend with xt loads.
                for kb in range(NKB):
                    nc.gpsimd.dma_start(wo_sb[:, kb * KB:(kb + 1) * KB, :],
                                        wo_e[:, kb * KB:(kb + 1) * KB, :])

            def wo_units(b, nch):
                """One token-half of wo for batch b: 4 output blocks × 32 k.
                Yields after each k-tile's 4 matmuls.  Evictions alternate
                scalar/vector and out-DMAs alternate scalar/sync so the final
                drain isn't serialized on one engine."""
                c0 = nch * QW
                psw = pp(4)
                agts = {}

                def issue_agt(kb):
                    t = agp.tile([P, KB, QW], BF16, name="agt")
                    nc.sync.dma_start(
                        t[:], ag_r[b][nch][:, kb * KB:(kb + 1) * KB, :])
                    agts[kb] = t

                issue_agt(0)
                issue_agt(1)
                for kb in range(NKB):
                    if kb + 2 < NKB:
                        issue_agt(kb + 2)
                    agt = agts.pop(kb)
                    for j in range(KB):
                        k = kb * KB + j
                        for mb in range(4):
                            nc.tensor.matmul(
                                psw[mb][:], wo_sb[:, k, mb * P:(mb + 1) * P],
                                agt[:, j, :],
                                start=(k == 0), stop=(k == KT - 1))
                        yield
                ows = []
                for mb in range(4):
                    ow = owp.tile([P, QW], BF16, name="ow")
                    if mb % 2 == 0:
                        nc.scalar.copy(ow[:], psw[mb][:])
                    else:
                        nc.vector.tensor_copy(ow[:], psw[mb][:])
                    ows.append(ow)
                for mb in range(4):
                    dma = nc.scalar.dma_start if mb % 2 == 0 else nc.sync.dma_start
                    dma(out_e[mb * P:(mb + 1) * P, b * S + c0:b * S + c0 + QW],
                        ows[mb][:])
                yield

            # ---- timeline ----
            preload()
            _drain(proj_units(0))
            finish(0)
            _merge(proj_units(1), attn_gen(0, 0), ratio_b=1.25)
            finish(1)
            gather(0, 0)
            wo_load()
            _merge(proj_units(2), attn_gen(0, 1), ratio_b=2.35)
            finish(2)
            gather(0, 1)
            _merge(proj_units(3), attn_gen(1, 0), ratio_b=1.25)
            finish(3)
            gather(1, 0)
            _merge(wo_units(0, 0), attn_gen(1, 1), ratio_b=2.5)
            gather(1, 1)
            _drain(wo_units(0, 1))
            _drain(wo_units(1, 0))
            _drain(wo_units(1, 1))

    nc.compile()
    return nc


def _get_compiled(variant):
    if variant not in _COMPILED:
        _COMPILED[variant] = _build(variant)
    return _COMPILED[variant]


def _detect_variant(mask2d):
    if not np.any(mask2d):
        return "nomask"
    tril = np.tril(mask2d)
    if not np.any(tril):
        iu = np.triu_indices(S, 1)
        if np.all(mask2d[iu] <= -1e8):
            return "causal"
    return "general"


def _pack_kt(w):
    """[R*128, N] -> [128, R, N] so that [:, k, :] is rows k*128..k*128+127."""
    return np.ascontiguousarray(w.reshape(w.shape[0] // P, P, -1).transpose(1, 0, 2))


def kernel(x, wq, wk, wv, wo, lora_q_a, lora_q_b, lora_v_a, lora_v_b,
           freqs_cos, freqs_sin, mask, start_pos=0, **_):
    global LAST_RESULTS
    bf = ml_dtypes.bfloat16
    x = np.asarray(x, np.float32)
    wq = np.asarray(wq, np.float32)
    wk = np.asarray(wk, np.float32)
    wv = np.asarray(wv, np.float32)
    wo = np.asarray(wo, np.float32)
    lora_q_a = np.asarray(lora_q_a, np.float32)
    lora_q_b = np.asarray(lora_q_b, np.float32)
    lora_v_a = np.asarray(lora_v_a, np.float32)
    lora_v_b = np.asarray(lora_v_b, np.float32)
    cos = np.asarray(freqs_cos, np.float32)
    sin = np.asarray(freqs_sin, np.float32)
    mask2d = np.asarray(mask, np.float32).reshape(S, S)

    variant = _detect_variant(mask2d)
    nc = _get_compiled(variant)

    # fold LoRA + scale; permute rope pairs (evens then odds within each head)
    wq_eff = (wq + lora_q_a @ lora_q_b) * np.float32(1.0 / np.sqrt(HD))
    wv_eff = wv + lora_v_a @ lora_v_b
    perm = np.concatenate([np.arange(0, HD, 2), np.arange(1, HD, 2)])
    qperm = (np.arange(H)[:, None] * HD + perm[None, :]).reshape(-1)
    kperm = (np.arange(KVH)[:, None] * HD + perm[None, :]).reshape(-1)
    wq_eff = wq_eff[:, qperm]
    wk_p = wk[:, kperm]

    xt = x.reshape(T, D).T                               # [4096, 2048]
    # [128, NQ, KT, QW]: per partition, each quarter's k-tiles contiguous.
    xt_p = np.ascontiguousarray(
        xt.reshape(KT, P, NQ, QW).transpose(1, 2, 0, 3)).astype(bf)
    c64 = np.tile(cos.T, (1, B))                        # [64, 2048]
    s64 = np.tile(sin.T, (1, B))
    cosT = np.concatenate([c64, c64], axis=0).astype(bf)   # [c; c]
    sinT = np.concatenate([s64, -s64], axis=0).astype(bf)  # [s; -s]

    if variant == "general":
        maskT = np.ascontiguousarray(mask2d.T)          # [st, sq]
        mk = _pack_kt(maskT).astype(bf)                 # [128, 8, 1024]
    else:
        mk = None

    in_maps = []
    for c in range(NCORES):
        im = {
            "xt": xt_p,
            "wq": _pack_kt(wq_eff[:, c * QCOLS:(c + 1) * QCOLS]).astype(bf),
            "wk": _pack_kt(wk_p[:, c * HD:(c + 1) * HD]).astype(bf),
            "wv": _pack_kt(wv_eff[:, c * HD:(c + 1) * HD]).astype(bf),
            "wo": _pack_kt(wo[:, c * QCOLS:(c + 1) * QCOLS]).astype(bf),
            "cos": cosT,
            "sin": sinT,
        }
        if mk is not None:
            im["mk"] = mk
        in_maps.append(im)

    res = run_bass_kernel_spmd(nc, in_maps, core_ids=list(range(NCORES)))
    LAST_RESULTS = res
    outT = np.concatenate(
        [np.asarray(res.results[c]["out"], dtype=np.float32)
         for c in range(NCORES)], axis=0)
    return np.ascontiguousarray(outT.T).reshape(B, S, D)


# revision 28
# speedup vs baseline: 1.1173x; 1.1173x over previous
"""Distributed Trainium2 Bass kernel for nn_Attention (GQA attention + LoRA + RoPE).

Sharding: tensor-parallel over heads across 8 NeuronCores.
  - core c owns Q heads 4c..4c+3 and KV head c (GQA group).
  - wq/wk/wv column-sharded; wo COLUMN-sharded (each core computes a
    512-column slice of the output over the full 4096 contraction, fed by an
    AllGather of all cores' per-head attention outputs).
  - LoRA is folded into wq/wv on the host (x@wq + (x@A)@B == x@(wq + A@B)).
  - 1/sqrt(HD) folded into wq.
  - RoPE pair permutation folded into wq/wk column order: within each head the
    even dims come first, odd dims second, so on-device RoPE is plain
    elementwise math on partition halves.

Schedule: software-pipelined emission.  The tensor queue is in-order, so
attention matmuls (which wait on scalar-engine exp) are interleaved at
emission time with the next projection quarter / wo matmuls; exp latency
hides under independent matmuls instead of head-of-line blocking.
DMAs are batched (multi-k-tile, contiguous per partition) so the issue
queues don't bottleneck.  Causal masking of diagonal tiles is a vector
multiply with precomputed 0/1 tiles (gpsimd must stay free: it hosts the
blocking AllGather waits).  Output is written bf16 and widened on host.
"""

import sys
import types

import numpy as np
import ml_dtypes

import concourse.bass as bass
from concourse import bacc
import concourse.mybir as mybir
import concourse.tile as tile
from concourse.bass_utils import run_bass_kernel_spmd
from concourse.masks import make_identity


def _ensure_axon_hooks():
    """run_bass_kernel_spmd(trace=True) imports antenv.axon_hooks, which some
    images lack; install a no-op shim so a BASS_TRACE env var can't crash us."""
    try:
        import antenv
    except ImportError:
        return
    if "antenv.axon_hooks" in sys.modules:
        return
    try:
        from antenv import axon_hooks  # noqa: F401
        return
    except ImportError:
        pass
    mod = types.ModuleType("antenv.axon_hooks")
    mod._hook = None
    mod.set_axon_ntff_profile_hook = lambda h: setattr(mod, "_hook", h)
    mod.get_axon_ntff_profile_hook = lambda: mod._hook
    sys.modules["antenv.axon_hooks"] = mod
    antenv.axon_hooks = mod


_ensure_axon_hooks()

B, S, D = 2, 1024, 4096
H, KVH, HD = 32, 8, 128
NCORES = 8
HPC = H // NCORES            # 4 q heads per core
QCOLS = HPC * HD             # 512
T = B * S                    # 2048
P = 128
KT = D // P                  # 32 k tiles
KB = 4                       # k tiles per DMA block
NKB = KT // KB               # 8 blocks
NQ = 4                       # token quarters (512 tokens each)
QW = T // NQ                 # 512
SQC = 2                      # sq chunks per batch
STB = S // P                 # 8 st blocks per batch

FP32 = mybir.dt.float32
BF16 = mybir.dt.bfloat16
EXP = mybir.ActivationFunctionType.Exp

_COMPILED = {}
LAST_RESULTS = None


def _st_list(variant, sqc):
    """st blocks contributing to sq chunk sqc.
    Returns (st, flag): causal -> flag means diagonal-crossing (needs
    triangular zeroing of probs); general -> flag means mask preload."""
    out = []
    for st in range(STB):
        if variant == "causal":
            if st >= 4 * sqc + 4:
                continue  # fully masked
            flag = st >= 4 * sqc
        elif variant == "nomask":
            flag = False
        else:
            flag = True
        out.append((st, flag))
    return out


_DONE = object()


def _merge(gen_a, gen_b, offset=3, ratio_b=1.0):
    """Round-robin merge of two emission generators.  gen_a (dense matmul
    work) leads by `offset` units so gen_b's cross-engine deps are ready;
    ratio_b units of gen_b (cumulative, may be fractional) are pulled per
    gen_a unit."""
    for _ in range(offset):
        next(gen_a, _DONE)
    a_done = b_done = False
    owed = 0.0
    while not (a_done and b_done):
        if not a_done:
            a_done = next(gen_a, _DONE) is _DONE
        owed += ratio_b
        while owed >= 1.0:
            owed -= 1.0
            if not b_done:
                b_done = next(gen_b, _DONE) is _DONE


def _drain(gen):
    for _ in gen:
        pass


def _build(variant, debug=False):
    nc = bacc.Bacc(None)

    xt_e = nc.declare_dram_parameter("xt", [P, NQ, KT, QW], BF16, isOutput=False)
    wq_e = nc.declare_dram_parameter("wq", [P, KT, QCOLS], BF16, isOutput=False)
    wk_e = nc.declare_dram_parameter("wk", [P, KT, HD], BF16, isOutput=False)
    wv_e = nc.declare_dram_parameter("wv", [P, KT, HD], BF16, isOutput=False)
    wo_e = nc.declare_dram_parameter("wo", [P, KT, QCOLS], BF16, isOutput=False)
    # cos: [c; c] duplicated halves.  sin: [s; -s] (negated bottom half).
    cos_e = nc.declare_dram_parameter("cos", [P, T], BF16, isOutput=False)
    sin_e = nc.declare_dram_parameter("sin", [P, T], BF16, isOutput=False)
    if variant == "general":
        mk_e = nc.declare_dram_parameter("mk", [P, STB, S], BF16, isOutput=False)
    out_e = nc.declare_dram_parameter("out", [QCOLS, T], BF16, isOutput=True)

    general = variant == "general"
    xt_bufs = 4 if general else 6
    pr_bufs = 8 if general else 10

    with tile.TileContext(nc) as tc:
        with (
            tc.tile_pool(name="wpool", bufs=1) as wpool,
            tc.tile_pool(name="cst", bufs=1) as cst,
            tc.tile_pool(name="persist", bufs=1) as persist,
            tc.tile_pool(name="xt", bufs=xt_bufs) as xtp,
            tc.tile_pool(name="ev", bufs=4) as evp,
            tc.tile_pool(name="rt", bufs=2) as rtp,
            tc.tile_pool(name="probs", bufs=pr_bufs) as prp,
            tc.tile_pool(name="dsum", bufs=5) as dsp,
            tc.tile_pool(name="misc", bufs=2) as mip,
            tc.tile_pool(name="ag", bufs=3) as agp,
            tc.tile_pool(name="ow", bufs=4) as owp,
            tc.tile_pool(name="ps", bufs=1, space="PSUM") as psp,
            tc.tile_pool(name="dram", bufs=1, space="DRAM") as dram,
        ):
            # ---- resident weights / constants ----
            wq_sb = wpool.tile([P, KT, QCOLS], BF16, name="wq_sb")
            wk_sb = wpool.tile([P, KT, HD], BF16, name="wk_sb")
            wv_sb = wpool.tile([P, KT, HD], BF16, name="wv_sb")
            wo_sb = wpool.tile([P, KT, QCOLS], BF16, name="wo_sb")
            cos_sb = wpool.tile([P, T], BF16, name="cos_sb")
            sin_sb = wpool.tile([P, T], BF16, name="sin_sb")
            if general:
                mk_sb = wpool.tile([P, STB, S], BF16, name="mk_sb")

            ident = cst.tile([P, P], BF16, name="ident")
            make_identity(nc, ident)
            ones_sq = cst.tile([P, P], BF16, name="ones_sq")
            nc.vector.memset(ones_sq[:], 1.0)
            # Causal diagonal mask for a PAIRED probs tile [st2c | st2c+1] over
            # one 256-query chunk: left half keeps q >= p, right keeps
            # q >= p+128.  The relative structure is identical for every
            # chunk, so one tile serves all diagonals.
            CW = QW // 2
            dmask2 = cst.tile([P, QW], BF16, name="dmask2")
            if variant == "causal":
                nc.gpsimd.memset(dmask2[:], 1.0)
                for j in range(2):
                    nc.gpsimd.affine_select(
                        out=dmask2[:, j * CW:(j + 1) * CW],
                        in_=dmask2[:, j * CW:(j + 1) * CW],
                        compare_op=mybir.AluOpType.is_ge,
                        fill=0.0, base=-j * P, channel_multiplier=-1,
                        pattern=[[1, CW]])

            # ---- persistent activations ----
            q_rot = [[persist.tile([P, S], BF16, name=f"q{h}_{b}")
                      for b in range(B)] for h in range(HPC)]
            k_rot = [persist.tile([P, S], BF16, name=f"k{b}") for b in range(B)]
            v_sb = [persist.tile([P, STB, P], BF16, name=f"v{b}") for b in range(B)]
            attn = [[persist.tile([P, S], BF16, name=f"attn{h}_{b}")
                     for b in range(B)] for h in range(HPC)]

            # one AllGather per (batch, token-half): each triggers as soon as
            # its attention chunk is done, so no gather is ever on the
            # critical path of the wo matmuls that consume it.
            ag_in = [[dram.tile([HPC * P, QW], BF16, name=f"agin{b}_{hf}")
                      for hf in range(SQC)] for b in range(B)]
            ag_out = [[dram.tile([H * P, QW], BF16, addr_space="Shared",
                                 name=f"agout{b}_{hf}")
                       for hf in range(SQC)] for b in range(B)]
            ag_r = [[ag_out[b][hf].rearrange("(k p) t -> p k t", p=P)
                     for hf in range(SQC)] for b in range(B)]

            def pp(n=1):
                return [psp.tile([P, QW], FP32, name="pp", tag="pp", bufs=6)
                        for _ in range(n)]

            def ap(shape=(P, QW), dtype=FP32):
                return psp.tile(list(shape), dtype, name="ap", tag="ap", bufs=2)

            xt_blocks = {}

            def issue_xt(qx, kb):
                t = xtp.tile([P, KB, QW], BF16, name="xt")
                nc.sync.dma_start(t[:], xt_e[:, qx, kb * KB:(kb + 1) * KB, :])
                xt_blocks[(qx, kb)] = t

            def rope(dst, dst_off, src_bf, qoff):
                """RoPE on split layout (a=0:64, b=64:128).
                p1 = [a*c; b*c];  p2 = [a*s; -b*s];  swap halves of p2;
                dst = p1 + p2sw = [a*c - b*s; a*s + b*c]."""
                c = cos_sb[:, qoff:qoff + QW]
                s = sin_sb[:, qoff:qoff + QW]
                p1 = rtp.tile([P, QW], BF16, name="p1")
                p2 = rtp.tile([P, QW], BF16, name="p2")
                p2sw = rtp.tile([P, QW], BF16, name="p2sw")
                nc.vector.tensor_mul(p1[:], src_bf[:], c)
                nc.vector.tensor_mul(p2[:], src_bf[:], s)
                nc.vector.tensor_copy(p2sw[0:64, :], p2[64:128, :])
                nc.vector.tensor_copy(p2sw[64:128, :], p2[0:64, :])
                nc.vector.tensor_add(dst[:, dst_off:dst_off + QW], p1[:], p2sw[:])

            def preload():
                """Startup DMA order: weights arrive in k order just ahead of
                the k-loop's consumption, so no single large transfer delays
                an early k-tile."""
                def wblk(w_sb, w_e, k0, k1):
                    nc.sync.dma_start(w_sb[:, k0:k1, :], w_e[:, k0:k1, :])

                # wk/wv whole (8 KB contiguous per partition — best DMA line
                # size); wq/xt per 4-k block so arrival tracks consumption.
                nc.sync.dma_start(wk_sb[:], wk_e[:])
                issue_xt(0, 0)
                nc.sync.dma_start(wv_sb[:], wv_e[:])
                wblk(wq_sb, wq_e, 0, KB)
                nc.gpsimd.dma_start(cos_sb[:], cos_e[:])
                nc.gpsimd.dma_start(sin_sb[:], sin_e[:])
                for kb in range(1, NKB):
                    wblk(wq_sb, wq_e, kb * KB, (kb + 1) * KB)
                    issue_xt(0, kb)
                if general:
                    nc.gpsimd.dma_start(mk_sb[:], mk_e[:])

            proj_ps = {}

            def proj_units(qx):
                """Yield after each k-tile's 6 matmuls.  Prefetches the next
                quarter's xt blocks as this quarter's are consumed."""
                toff = qx * QW
                psums = pp(6)
                proj_ps[qx] = psums
                for kb in range(NKB):
                    xt = xt_blocks.pop((qx, kb))
                    for j in range(KB):
                        k = kb * KB + j
                        rhs = xt[:, j, :]
                        for mb in range(6):
                            if mb == 0:
                                w = wk_sb[:, k, :]
                            elif mb == 1:
                                w = wv_sb[:, k, :]
                            else:
                                w = wq_sb[:, k, (mb - 2) * P:(mb - 1) * P]
                            nc.tensor.matmul(psums[mb][:], w, rhs,
                                             start=(k == 0), stop=(k == KT - 1))
                        yield
                    if qx + 1 < NQ:
                        issue_xt(qx + 1, kb)

            def finish(qx):
                """Evict + rope this quarter's projections.  Order: k first
                (scores need it), then q heads, then v (PV needs it last)."""
                b, boff = qx // 2, (qx % 2) * QW
                toff = qx * QW
                pk, pvv, *pq = proj_ps.pop(qx)
                ke = evp.tile([P, QW], BF16, name="ke", tag="qe")
                nc.scalar.copy(ke[:], pk[:])
                rope(k_rot[b], boff, ke, toff)
                for h in range(HPC):
                    qe = evp.tile([P, QW], BF16, name="qe", tag="qe")
                    nc.scalar.copy(qe[:], pq[h][:])
                    rope(q_rot[h][b], boff, qe, toff)
                ve = evp.tile([P, QW], BF16, name="ve", tag="qe")
                nc.scalar.copy(ve[:], pvv[:])
                for i in range(QW // P):
                    st = (qx % 2) * 4 + i
                    tp = ap((P, P), BF16)
                    nc.tensor.transpose(tp[:], ve[:, i * P:(i + 1) * P], ident[:])
                    nc.scalar.copy(v_sb[b][:, st, :], tp[:])

            def attn_units(b, sqc):
                """Yield at points where the tensor queue may have to wait on
                the scalar engine (exp) — merge partner fills the gaps.
                The softmax denominator is computed by tree-summing the probs
                tiles on the vector engine (keys are partition-aligned across
                st blocks), then ONE ones-matmul — instead of one matmul per
                tile."""
                sq0 = sqc * QW
                stl = _st_list(variant, sqc)
                n = len(stl)
                for h in range(HPC):
                    prtiles = []
                    sums = []

                    def vadd(x, y):
                        t = dsp.tile([P, QW], BF16, name="ds")
                        nc.vector.tensor_add(t[:], x[:], y[:])
                        return t

                    for i0 in range(0, n, 2):
                        for st, flag in stl[i0:i0 + 2]:
                            pss = ap()
                            preload_mk = general and flag
                            if preload_mk:
                                nc.tensor.matmul(pss[:], ident[:],
                                                 mk_sb[:, st, sq0:sq0 + QW],
                                                 start=True, stop=False)
                            nc.tensor.matmul(
                                pss[:], k_rot[b][:, st * P:(st + 1) * P],
                                q_rot[h][b][:, sq0:sq0 + QW],
                                start=(not preload_mk), stop=True)
                            pr = prp.tile([P, QW], BF16, name="pr", tag="pr")
                            nc.scalar.activation(pr[:], pss[:], EXP)
                            if flag and variant == "causal":
                                j = st - 4 * sqc
                                nc.vector.tensor_mul(pr[:], pr[:],
                                                     dmask[:, j, :])
                            prtiles.append(pr)
                        sums.append(vadd(prtiles[-2], prtiles[-1]))
                        yield
                    pso = ap()
                    psdb = ap()
                    for i0 in range(0, n, 2):
                        for i in range(i0, min(i0 + 2, n)):
                            st = stl[i][0]
                            nc.tensor.matmul(pso[:], v_sb[b][:, st, :],
                                             prtiles[i][:],
                                             start=(i == 0), stop=(i == n - 1))
                        if len(sums) > 1:
                            sums = sums[2:] + [vadd(sums[0], sums[1])]
                        yield
                    nc.tensor.matmul(psdb[:], ones_sq[:], sums[0][:],
                                     start=True, stop=True)
                    rb = mip.tile([P, QW], FP32, name="rb")
                    nc.vector.reciprocal_approx_fast(rb[:], psdb[:])
                    nc.vector.tensor_mul(attn[h][b][:, sq0:sq0 + QW],
                                         pso[:], rb[:])
                    nc.gpsimd.dma_start(ag_in[b][sqc][h * P:(h + 1) * P, :],
                                        attn[h][b][:, sq0:sq0 + QW])
                    yield

            def attn_units_causal(b, half):
                """Block-exact causal attention at 256-query granularity.
                Two 256-wide score tiles for adjacent key blocks share one
                PSUM bank side-by-side, so exp still runs on full [128,512]
                tiles and the diagonal needs a single mask multiply.
                half 0 covers chunks 0-1 (keys st 0..3, from the even
                quarter); half 1 covers chunks 2-3 (st 0..7)."""
                for h in range(HPC):
                    for c in (2 * half, 2 * half + 1):
                        q0c = c * CW
                        qs = q_rot[h][b][:, q0c:q0c + CW]
                        npair = c + 1
                        prtiles = []
                        sums = []

                        def vadd(x, y):
                            t = dsp.tile([P, QW], BF16, name="ds")
                            nc.vector.tensor_add(t[:], x[:], y[:])
                            return t

                        for pi in range(npair):
                            pss = ap()
                            for sub in range(2):
                                st = 2 * pi + sub
                                nc.tensor.matmul(
                                    pss[:, sub * CW:(sub + 1) * CW],
                                    k_rot[b][:, st * P:(st + 1) * P], qs,
                                    start=True, stop=True)
                            pr = prp.tile([P, QW], BF16, name="pr", tag="pr")
                            nc.scalar.activation(pr[:], pss[:], EXP)
                            if pi == npair - 1:
                                nc.vector.tensor_mul(pr[:], pr[:], dmask2[:])
                            prtiles.append(pr)
                            yield
                        pso = ap((P, CW))
                        psdb = ap((P, CW))
                        for pi in range(npair):
                            for sub in range(2):
                                st = 2 * pi + sub
                                nc.tensor.matmul(
                                    pso[:], v_sb[b][:, st, :],
                                    prtiles[pi][:, sub * CW:(sub + 1) * CW],
                                    start=(pi == 0 and sub == 0),
                                    stop=(pi == npair - 1 and sub == 1))
                            if pi % 2 == 1:
                                sums.append(vadd(prtiles[pi - 1], prtiles[pi]))
                            yield
                        if npair % 2 == 1:
                            sums.append(prtiles[-1])
                        while len(sums) > 1:
                            sums = sums[2:] + [vadd(sums[0], sums[1])]
                        sh = dsp.tile([P, CW], BF16, name="dsh")
                        nc.vector.tensor_add(sh[:], sums[0][:, 0:CW],
                                             sums[0][:, CW:QW])
                        nc.tensor.matmul(psdb[:], ones_sq[:], sh[:],
                                         start=True, stop=True)
                        rb = mip.tile([P, CW], FP32, name="rbc", tag="rbc")
                        nc.vector.reciprocal_approx_fast(rb[:], psdb[:])
                        nc.vector.tensor_mul(attn[h][b][:, q0c:q0c + CW],
                                             pso[:], rb[:])
                        yield
                    nc.gpsimd.dma_start(
                        ag_in[b][half][h * P:(h + 1) * P, :],
                        attn[h][b][:, half * QW:(half + 1) * QW])
                    yield

            def attn_gen(b, half):
                if variant == "causal":
                    return attn_units_causal(b, half)
                return attn_units(b, half)

            def gather(b, hf):
                nc.gpsimd.collective_compute(
                    "AllGather", mybir.AluOpType.bypass,
                    ins=[ag_in[b][hf][:].opt()],
                    outs=[ag_out[b][hf][:].opt()],
                    replica_groups=[list(range(NCORES))],
                )

            def wo_load():
                # gpsimd queue is idle here (and only hosts the collectives
                # later), so weight streaming doesn't contend with xt loads.
                for kb in range(NKB):
                    nc.gpsimd.dma_start(wo_sb[:, kb * KB:(kb + 1) * KB, :],
                                        wo_e[:, kb * KB:(kb + 1) * KB, :])

            def wo_units(b, nch):
                """One token-half of wo for batch b: 4 output blocks × 32 k.
                Yields after each k-tile's 4 matmuls.  Evictions alternate
                scalar/vector and out-DMAs alternate scalar/sync so the final
                drain isn't serialized on one engine."""
                c0 = nch * QW
                psw = pp(4)
                agts = {}

                def issue_agt(kb):
                    t = agp.tile([P, KB, QW], BF16, name="agt")
                    nc.sync.dma_start(
                        t[:], ag_r[b][nch][:, kb * KB:(kb + 1) * KB, :])
                    agts[kb] = t

                issue_agt(0)
                issue_agt(1)
                for kb in range(NKB):
                    if kb + 2 < NKB:
                        issue_agt(kb + 2)
                    agt = agts.pop(kb)
                    for j in range(KB):
                        k = kb * KB + j
                        for mb in range(4):
                            nc.tensor.matmul(
                                psw[mb][:], wo_sb[:, k, mb * P:(mb + 1) * P],
                                agt[:, j, :],
                                start=(k == 0), stop=(k == KT - 1))
                        yield
                ows = []
                for mb in range(4):
                    ow = owp.tile([P, QW], BF16, name="ow")
                    if mb % 2 == 0:
                        nc.scalar.copy(ow[:], psw[mb][:])
                    else:
                        nc.vector.tensor_copy(ow[:], psw[mb][:])
                    ows.append(ow)
                for mb in range(4):
                    dma = nc.scalar.dma_start if mb % 2 == 0 else nc.sync.dma_start
                    dma(out_e[mb * P:(mb + 1) * P, b * S + c0:b * S + c0 + QW],
                        ows[mb][:])
                yield

            # ---- timeline ----
            preload()
            _drain(proj_units(0))
            finish(0)
            _merge(proj_units(1), attn_gen(0, 0), ratio_b=1.25)
            finish(1)
            gather(0, 0)
            wo_load()
            _merge(proj_units(2), attn_gen(0, 1), ratio_b=2.35)
            finish(2)
            gather(0, 1)
            _merge(proj_units(3), attn_gen(1, 0), ratio_b=1.25)
            finish(3)
            gather(1, 0)
            _merge(wo_units(0, 0), attn_gen(1, 1), ratio_b=2.5)
            gather(1, 1)
            _drain(wo_units(0, 1))
            _drain(wo_units(1, 0))
            _drain(wo_units(1, 1))

    nc.compile()
    return nc


def _get_compiled(variant):
    if variant not in _COMPILED:
        _COMPILED[variant] = _build(variant)
    return _COMPILED[variant]


def _detect_variant(mask2d):
    if not np.any(mask2d):
        return "nomask"
    tril = np.tril(mask2d)
    if not np.any(tril):
        iu = np.triu_indices(S, 1)
        if np.all(mask2d[iu] <= -1e8):
            return "causal"
    return "general"


def _pack_kt(w):
    """[R*128, N] -> [128, R, N] so that [:, k, :] is rows k*128..k*128+127."""
    return np.ascontiguousarray(w.reshape(w.shape[0] // P, P, -1).transpose(1, 0, 2))


def kernel(x, wq, wk, wv, wo, lora_q_a, lora_q_b, lora_v_a, lora_v_b,
           freqs_cos, freqs_sin, mask, start_pos=0, **_):
    global LAST_RESULTS
    bf = ml_dtypes.bfloat16
    x = np.asarray(x, np.float32)
    wq = np.asarray(wq, np.float32)
    wk = np.asarray(wk, np.float32)
    wv = np.asarray(wv, np.float32)
    wo = np.asarray(wo, np.float32)
    lora_q_a = np.asarray(lora_q_a, np.float32)
    lora_q_b = np.asarray(lora_q_b, np.float32)
    lora_v_a = np.asarray(lora_v_a, np.float32)
    lora_v_b = np.asarray(lora_v_b, np.float32)
    cos = np.asarray(freqs_cos, np.float32)
    sin = np.asarray(freqs_sin, np.float32)
    mask2d = np.asarray(mask, np.float32).reshape(S, S)

    variant = _detect_variant(mask2d)
    nc = _get_compiled(variant)

    # fold LoRA + scale; permute rope pairs (evens then odds within each head)
    wq_eff = (wq + lora_q_a @ lora_q_b) * np.float32(1.0 / np.sqrt(HD))
    wv_eff = wv + lora_v_a @ lora_v_b
    perm = np.concatenate([np.arange(0, HD, 2), np.arange(1, HD, 2)])
    qperm = (np.arange(H)[:, None] * HD + perm[None, :]).reshape(-1)
    kperm = (np.arange(KVH)[:, None] * HD + perm[None, :]).reshape(-1)
    wq_eff = wq_eff[:, qperm]
    wk_p = wk[:, kperm]

    xt = x.reshape(T, D).T                               # [4096, 2048]
    # [128, NQ, KT, QW]: per partition, each quarter's k-tiles contiguous.
    xt_p = np.ascontiguousarray(
        xt.reshape(KT, P, NQ, QW).transpose(1, 2, 0, 3)).astype(bf)
    c64 = np.tile(cos.T, (1, B))                        # [64, 2048]
    s64 = np.tile(sin.T, (1, B))
    cosT = np.concatenate([c64, c64], axis=0).astype(bf)   # [c; c]
    sinT = np.concatenate([s64, -s64], axis=0).astype(bf)  # [s; -s]

    if variant == "general":
        maskT = np.ascontiguousarray(mask2d.T)          # [st, sq]
        mk = _pack_kt(maskT).astype(bf)                 # [128, 8, 1024]
    else:
        mk = None

    in_maps = []
    for c in range(NCORES):
        im = {
            "xt": xt_p,
            "wq": _pack_kt(wq_eff[:, c * QCOLS:(c + 1) * QCOLS]).astype(bf),
            "wk": _pack_kt(wk_p[:, c * HD:(c + 1) * HD]).astype(bf),
            "wv": _pack_kt(wv_eff[:, c * HD:(c + 1) * HD]).astype(bf),
            "wo": _pack_kt(wo[:, c * QCOLS:(c + 1) * QCOLS]).astype(bf),
            "cos": cosT,
            "sin": sinT,
        }
        if mk is not None:
            im["mk"] = mk
        in_maps.append(im)

    res = run_bass_kernel_spmd(nc, in_maps, core_ids=list(range(NCORES)))
    LAST_RESULTS = res
    outT = np.concatenate(
        [np.asarray(res.results[c]["out"], dtype=np.float32)
         for c in range(NCORES)], axis=0)
    return np.ascontiguousarray(outT.T).reshape(B, S, D)
